# revision 28
# baseline (speedup 1.0000x reference)
"""Trainium2 Bass kernel for nn_AutoEncoder (per-dim autoencoder bmm pair).

Math (per embedding dim d):
    C[d]     = E[d] @ w[d]      E[d]: [400, 4000],  w[d] = word_embeddings[:, d]
    W_out[d] = D[d] @ C[d]      D[d]: [4000, 400]
Returns (W_out, C).

Sharding: the 64 dims are embarrassingly parallel -> 8 dims per NeuronCore.

Device strategy (per core, 8 dims):
  - bmm1 on the TensorEngine. E is host-transposed to E_t[d] = [4096, 400]
    (word-major, padded 4000->4096) so the contraction (words) lands on the
    SBUF partition axis: 32 accumulating matmuls of lhsT=w_chunk [128,1],
    rhs=E_t chunk [128,400] -> PSUM C [1,400].
  - C is broadcast across partitions with a rank-1 matmul (ones [1,128] x
    C_row [1,400] -> PSUM [128,400]).
  - bmm2 on the VectorEngine with D in its natural [4000, 400] layout:
    fused tensor_tensor_reduce (mult + add-reduce over classes) per
    128-word chunk -> W_out column [128,1].
  - W_out columns [128,32] go through the DVE 32x32 block transpose; the
    resulting layout maps each partition row to 32 contiguous words, so the
    store is a clean strided DMA.
All heavy DMA uses full 128-partition tiles with >=1600B contiguous rows.
"""

import numpy as np
from contextlib import ExitStack

import concourse.bass as bass
import concourse.bacc as bacc
import concourse.mybir as mybir
import concourse.tile as tile
from concourse.bass_utils import run_bass_kernel_spmd

def _install_ntff_shim():
    """Provide antenv.axon_hooks (absent in this container) so
    run_bass_kernel_spmd(trace=True) can drive NTFF profiling via the
    libaxon ctypes ABI. No-op if already present or the .so lacks the
    profiling symbols."""
    import sys, types, contextlib, ctypes

    try:
        import antenv.axon_hooks  # noqa: F401
        return
    except ImportError:
        pass
    try:
        import antenv
    except ImportError:
        return
    mod = types.ModuleType("antenv.axon_hooks")
    state = {"hook": None}
    mod.set_axon_ntff_profile_hook = lambda h: state.__setitem__("hook", h)
    mod.get_axon_ntff_profile_hook = lambda: state["hook"]
    sys.modules["antenv.axon_hooks"] = mod
    antenv.axon_hooks = mod
    try:
        lib = ctypes.CDLL("/opt/axon/libaxon_pjrt.so")
    except OSError:
        return
    if not hasattr(lib, "axon_start_nrt_profile"):
        return
    lib.axon_start_nrt_profile.argtypes = [
        ctypes.POINTER(ctypes.c_int64),
        ctypes.c_size_t,
    ]
    lib.axon_start_nrt_profile.restype = ctypes.c_int64
    lib.axon_stop_nrt_profile.argtypes = [ctypes.c_char_p]
    lib.axon_stop_nrt_profile.restype = ctypes.c_int64

    @contextlib.contextmanager
    def _hook(output_dir, device_ids):
        import jax

        jax.devices()
        if device_ids:
            ids = (ctypes.c_int64 * len(device_ids))(*device_ids)
            rc = lib.axon_start_nrt_profile(ids, len(device_ids))
        else:
            rc = lib.axon_start_nrt_profile(None, 0)
        if rc != 0:
            raise RuntimeError(f"axon_start_nrt_profile rc={rc}")
        try:
            yield
        finally:
            n = lib.axon_stop_nrt_profile(str(output_dir).encode())
            print(f"profile: {n} file(s) written to {output_dir}", file=sys.stderr)

    state["hook"] = _hook


_install_ntff_shim()

F32 = mybir.dt.float32

N_CORES = 8
DIMS = 64
DPC = DIMS // N_CORES  # dims per core
NW = 4000              # words
NWP = 4096             # padded words (32 chunks of 128)
NCLS = 400             # classes
NCHUNK = NWP // 128    # 32 word chunks per dim


def build_module() -> bass.Bass:
    nc = bacc.Bacc()
    Et = nc.dram_tensor("Et", [DPC, NW, NCLS], F32, kind="ExternalInput")
    Dn = nc.dram_tensor("Dn", [DPC, NW, NCLS], F32, kind="ExternalInput")
    Wv = nc.dram_tensor("Wv", [128, DPC * NCHUNK], F32, kind="ExternalInput")
    Co = nc.dram_tensor("Co", [DPC, NCLS], F32, kind="ExternalOutput")
    Wo = nc.dram_tensor("Wo", [DPC, NW], F32, kind="ExternalOutput")

    with tile.TileContext(nc) as tc, ExitStack() as ctx:
        const_pool = ctx.enter_context(tc.tile_pool(name="const", bufs=1))
        e_pool = ctx.enter_context(tc.tile_pool(name="e", bufs=8))
        d_pool = ctx.enter_context(tc.tile_pool(name="d", bufs=8))
        dmid_pool = ctx.enter_context(tc.tile_pool(name="dmid", bufs=3))
        dtail_pool = ctx.enter_context(tc.tile_pool(name="dtail", bufs=3))
        crow_pool = ctx.enter_context(tc.tile_pool(name="crow", bufs=3))
        cb_pool = ctx.enter_context(tc.tile_pool(name="cb", bufs=3))
        scr_pool = ctx.enter_context(tc.tile_pool(name="scr", bufs=3))
        wcols_pool = ctx.enter_context(tc.tile_pool(name="wcols", bufs=3))
        wt_pool = ctx.enter_context(tc.tile_pool(name="wt", bufs=3))
        pc_pool = ctx.enter_context(tc.tile_pool(name="pc", bufs=3, space="PSUM"))
        pcb_pool = ctx.enter_context(tc.tile_pool(name="pcb", bufs=3, space="PSUM"))

        ones = const_pool.tile([1, 128], F32)
        nc.gpsimd.memset(ones[:], 1.0)
        wv = const_pool.tile([128, DPC * NCHUNK], F32)
        nc.sync.dma_start(wv[:], Wv[:, :])

        mult = mybir.AluOpType.mult
        add = mybir.AluOpType.add

        def emit_bmm1(d):
            # ---- bmm1: C[d,c] = sum_w E_t[d,w,c] * w_d[w]  (PE) ----
            cp = pc_pool.tile([1, NCLS], F32)
            et_r = Et[d, : 28 * 128].rearrange("(g b p) c -> g p b c", b=4, p=128)
            for g in range(7):
                et = e_pool.tile([128, 4 * NCLS], F32)
                nc.sync.dma_start(et[:].rearrange("p (b c) -> p b c", b=4), et_r[g])
                for b in range(4):
                    j = 4 * g + b
                    nc.tensor.matmul(
                        cp[:],
                        wv[:, d * NCHUNK + j : d * NCHUNK + j + 1],
                        et[:, b * NCLS : (b + 1) * NCLS],
                        start=(j == 0),
                        stop=False,
                    )
            # words 3584..3968: 3 chunks of 128
            em = dmid_pool.tile([128, 3 * NCLS], F32)
            nc.sync.dma_start(
                em[:].rearrange("p (b c) -> p b c", b=3),
                Et[d, 28 * 128 : 31 * 128].rearrange("(b p) c -> p b c", p=128),
            )
            for b in range(3):
                j = 28 + b
                nc.tensor.matmul(
                    cp[:],
                    wv[:, d * NCHUNK + j : d * NCHUNK + j + 1],
                    em[:, b * NCLS : (b + 1) * NCLS],
                    start=False,
                    stop=False,
                )
            # tail words 3968..4000 (32 words, k=32)
            etl = dtail_pool.tile([32, NCLS], F32)
            nc.sync.dma_start(etl[:], Et[d, 31 * 128 :])
            nc.tensor.matmul(
                cp[:],
                wv[0:32, d * NCHUNK + 31 : d * NCHUNK + 32],
                etl[:],
                start=False,
                stop=True,
            )
            crow = crow_pool.tile([1, NCLS], F32)
            nc.scalar.copy(crow[:], cp[:])
            # outputs go on the scalar-engine HWDGE ring so the sync ring
            # streams inputs without head-of-line blocking on compute
            nc.scalar.dma_start(Co[d], crow[0:1, :])

            # broadcast C across all 128 partitions: cb[p,c] = C[c]
            cbp = pcb_pool.tile([128, NCLS], F32)
            nc.tensor.matmul(cbp[:], ones[:], crow[:], start=True, stop=True)
            cb = cb_pool.tile([128, NCLS], F32)
            nc.scalar.copy(cb[:], cbp[:])
            return cb

        def emit_bmm2(d, cb):
            # ---- bmm2: W_out[d,w] = sum_c D[d,w,c] * C[c]  (DVE) ----
            wcols = wcols_pool.tile([128, 32], F32)
            nc.vector.memset(wcols[:], 0.0)
            dn_r = Dn[d, : 28 * 128].rearrange("(g b p) c -> g p b c", b=4, p=128)
            for g in range(7):
                dt_ = d_pool.tile([128, 4 * NCLS], F32)
                nc.sync.dma_start(dt_[:].rearrange("p (b c) -> p b c", b=4), dn_r[g])
                for b in range(4):
                    j = 4 * g + b
                    scr = scr_pool.tile([128, NCLS], F32)
                    nc.vector.scalar_tensor_tensor(
                        out=scr[:],
                        in0=dt_[:, b * NCLS : (b + 1) * NCLS],
                        scalar=1.0,
                        in1=cb[:],
                        op0=mult,
                        op1=mult,
                        accum_out=wcols[:, j : j + 1],
                    )
            # words 3584..3968: 3 chunks of 128
            dm = dmid_pool.tile([128, 3 * NCLS], F32)
            nc.sync.dma_start(
                dm[:].rearrange("p (b c) -> p b c", b=3),
                Dn[d, 28 * 128 : 31 * 128].rearrange("(b p) c -> p b c", p=128),
            )
            for b in range(3):
                j = 28 + b
                scr = scr_pool.tile([128, NCLS], F32)
                nc.vector.scalar_tensor_tensor(
                    out=scr[:],
                    in0=dm[:, b * NCLS : (b + 1) * NCLS],
                    scalar=1.0,
                    in1=cb[:],
                    op0=mult,
                    op1=mult,
                    accum_out=wcols[:, j : j + 1],
                )
            # tail words 3968..4000 (32 words)
            dtl = dtail_pool.tile([32, NCLS], F32)
            nc.sync.dma_start(dtl[:], Dn[d, 31 * 128 :])
            scr = scr_pool.tile([128, NCLS], F32)
            nc.vector.scalar_tensor_tensor(
                out=scr[0:32, :],
                in0=dtl[:],
                scalar=1.0,
                in1=cb[0:32, :],
                op0=mult,
                op1=mult,
                accum_out=wcols[0:32, 31:32],
            )

            # DVE 32x32 block transpose: wt[32*bi + a, b] = wcols[32*bi + b, a]
            #   = W_out word a*128 + 32*bi + b  -> each row is 32 contiguous words
            wt = wt_pool.tile([128, 32], F32)
            nc.vector.transpose(wt[:], wcols[:])
            wo_g = Wo[d, : 31 * 128].rearrange("(a c) -> a c", c=128)  # [31, 128]
            for bi in range(4):
                nc.scalar.dma_start(
                    wo_g[:, bi * 32 : (bi + 1) * 32],
                    wt[32 * bi : 32 * bi + 31, :],
                )
            # tail words 3968..3999 live in wt partition 31 (a=31, bi=0)
            nc.scalar.dma_start(Wo[d, 31 * 128 :], wt[31:32, 0:32])

        # software pipeline: bmm2 runs one dim behind bmm1 so the final D
        # loads stream into an already-ready cb (no compute tail after the
        # last input DMA).
        cbs = {}
        cbs[0] = emit_bmm1(0)
        for d in range(1, DPC):
            cbs[d] = emit_bmm1(d)
            emit_bmm2(d - 1, cbs[d - 1])
        emit_bmm2(DPC - 1, cbs[DPC - 1])

    nc.compile()
    return nc


_CACHE: dict = {}


def _get_module() -> bass.Bass:
    if "nc" not in _CACHE:
        _CACHE["nc"] = build_module()
    return _CACHE["nc"]


def make_in_maps(word_embeddings, E, D):
    word_embeddings = np.asarray(word_embeddings, dtype=np.float32)
    E = np.asarray(E, dtype=np.float32)
    D = np.asarray(D, dtype=np.float32)
    in_maps = []
    for c in range(N_CORES):
        sl = slice(c * DPC, (c + 1) * DPC)
        et = np.ascontiguousarray(E[sl].transpose(0, 2, 1))
        dn = np.ascontiguousarray(D[sl])
        wpad = np.zeros((DPC, NWP), np.float32)
        wpad[:, :NW] = word_embeddings[:, sl].T
        wvv = np.ascontiguousarray(
            wpad.reshape(DPC, NCHUNK, 128).transpose(2, 0, 1)
        ).reshape(128, DPC * NCHUNK)
        in_maps.append({"Et": et, "Dn": dn, "Wv": wvv})
    return in_maps


def run(word_embeddings, E, D, **spmd_kwargs):
    nc = _get_module()
    in_maps = make_in_maps(word_embeddings, E, D)
    res = run_bass_kernel_spmd(nc, in_maps, core_ids=list(range(N_CORES)), **spmd_kwargs)
    W_out = np.concatenate([r["Wo"] for r in res.results], axis=0)
    C = np.concatenate([r["Co"] for r in res.results], axis=0)
    return (W_out, C), res


def kernel(word_embeddings, E, D, class_ids=None):
    (W_out, C), _ = run(word_embeddings, E, D)
    return (W_out, C)
